# revision 42
# baseline (speedup 1.0000x reference)
"""Trainium2 Bass kernel for nn_BasicModel_4054449127788.

Quantum-circuit product-state model: per-(batch, qubit) single-qubit gate
chain (Rx/Rz/Rx + data-encoding Rx, 6 blocks), then Z^(x)n expectation of
the kron-folded wavefunction.

Math used on device: adjacent Rx gates commute and merge (Rx(a)Rx(b) =
Rx(a+b)), collapsing the 24-gate chain to 13 gates.  The Z^(x)n
expectation of a product state factorizes exactly:
    O_b = prod_q (|s_{b,q,0}|^2 - |s_{b,q,1}|^2)
which is numerically *closer* to the reference's f32 kron-fold + signed
sum than an independently-rounded fold replica would be (the fold's own
f32 cancellation noise dominates: ~2e-3 normwise).

Sharding: pure data parallelism — batch 32 split 4-per-core across 8
NeuronCores, no cross-core communication.

Layout on device: 128 partitions = (b_local=4) x (32-lane quadrant), with
qubit q = 0..19 at partition b*32 + q (lanes 20..31 idle).  The state is
4 f32 components [s0r, s0i, s1r, s1i] in the free dim.  Each gate is two
DVE instructions:
    tmp  = perm(S) * (sin * sign_pattern)      (tensor_tensor)
    S'   = (S * cos_perpartition) + tmp        (scalar_tensor_tensor)
where perm is free-dim reversal (Rx) or within-pair swap (Rz).  The
product over the 20 qubits runs in-layout with stream_shuffle quadrant
rotations (no transpose DMA), and O rides in the same output DMA as the
state.
"""

import os
import numpy as np

_B = 32          # full batch
_Q = 20          # qubits
_NCORES = 8
_BL = _B // _NCORES   # batch per core = 4
_P = 128              # partitions: b_local * 32 + q
_COLS = (0, 1, 2, 5, 6, 7)
_HALF_PI = float(np.pi / 2)

_CACHE = {}

# Exposed for test harnesses: exec time of the last traced run (ns).
LAST_EXEC_TIME_NS = None
LAST_RESULTS = None


def _build_nc():
    import concourse.bass as bass
    import concourse.mybir as mybir
    from concourse.tile import TileContext

    f32 = mybir.dt.float32
    ADD = mybir.AluOpType.add
    MULT = mybir.AluOpType.mult
    MIN = mybir.AluOpType.min
    MAX = mybir.AluOpType.max
    SUB = mybir.AluOpType.subtract
    SIN = mybir.ActivationFunctionType.Sin

    nc = bass.Bass("TRN2", target_bir_lowering=False, debug=False)

    # single packed input: cols 0..24 raw gate angles (col 24 zero pad so
    # every merged alpha is a sum of three stride-4 columns), cols 25..80
    # sign patterns
    inp = nc.dram_tensor("inp", [_P, 81], f32, kind="ExternalInput")
    # state output (contiguous) + tiny O output (4 lanes)
    outp = nc.dram_tensor("outp", [_P, 4], f32, kind="ExternalOutput")
    o_out = nc.dram_tensor("o_out", [_BL, 1], f32, kind="ExternalOutput")

    def rot_mask(n):
        return [(i + n) % 32 for i in range(32)]

    with TileContext(nc) as tc:
        with (
            tc.tile_pool(name="cst", bufs=1) as cst,
            tc.tile_pool(name="ping", bufs=2) as ping,
        ):
            # split input DMAs: the small angle block lands first and
            # unblocks DVE prep; the pattern block is only needed later
            # by the SS build
            INA = cst.tile([_P, 25], f32, tag="INA")
            INP = cst.tile([_P, 56], f32, tag="INP")
            nc.sync.dma_start(out=INA[:], in_=inp[:, 0:25])
            nc.sync.dma_start(out=INP[:], in_=inp[:, 25:81])
            A = INA[:]
            PT = INP[:]

            # one workspace tile for all DVE scratch (fewer tile sems ->
            # shorter kernel-tail semaphore-reset sequence)
            WS = cst.tile([_P, 200], f32, tag="WS")
            T6 = WS[:, 0:6]
            T6b = WS[:, 6:12]
            AH = WS[:, 18:44]    # [sin half-angles | cos half-angles]
            U = WS[:, 44:70]
            R = WS[:, 70:96]
            RD = WS[:, 96:122]
            CS = WS[:, 122:148]  # [sin | cos] results
            SS = WS[:, 148:200]  # sin * per-comp sign pattern, comp-major

            OUT = cst.tile([_P, 4], f32, tag="OUT")

            # ---- merge the 24 raw gate angles into 13 half-angles ------
            # gate order g=0..12: Rx(a0), then 6x [Rz(beta_i), Rx(alpha_i)]
            # alpha_i = A[4i-2] + A[4i-1] + A[4i]  (A[24] = 0 pad makes the
            # i=6 group uniform), beta_i = A[4i-3], a0 = A[0].
            PI = float(np.pi)
            MAGIC = float(1.5 * 2.0 ** 23)
            nc.vector.tensor_tensor(T6b[:, :], A[:, 2:23:4], A[:, 3:24:4], ADD)
            nc.vector.tensor_tensor(T6[:, :], T6b[:, :], A[:, 4:25:4], ADD)
            nc.vector.tensor_scalar(AH[:, 0:1], A[:, 0:1], 0.5, None, MULT)
            nc.vector.tensor_scalar(AH[:, 1:12:2], A[:, 1:22:4], 0.5, None, MULT)
            nc.vector.tensor_scalar(AH[:, 2:13:2], T6[:, :], 0.5, None, MULT)

            # ---- cos/sin of half-angles --------------------------------
            # HW Sin needs args in [-pi, pi]; merged angles can exceed it.
            # Range-reduce t -> t - 2pi*round(t/2pi) using the f32
            # magic-constant round (x + 1.5*2^23 - 1.5*2^23), then clamp.
            nc.vector.tensor_scalar(
                AH[:, 13:26], AH[:, 0:13], _HALF_PI, None, ADD
            )
            nc.vector.tensor_scalar(
                U[:, :], AH[:, :], 1.0 / (2.0 * PI), MAGIC, MULT, ADD
            )
            nc.vector.tensor_scalar(R[:, :], U[:, :], MAGIC, None, SUB)
            nc.vector.scalar_tensor_tensor(
                RD[:, :], R[:, :], -2.0 * PI, AH[:, :], MULT, ADD
            )
            nc.vector.tensor_scalar(RD[:, :], RD[:, :], PI, -PI, MIN, MAX)
            nc.scalar.activation(CS[:, :], RD[:, :], SIN, bias=0.0, scale=1.0)
            SN = CS[:, 0:13]
            C = CS[:, 13:26]

            # ---- sin * per-component sign pattern ----------------------
            # comp-major layout SS[p, c*13 + g]; one TT with a broadcast AP
            sn_b = SN.unsqueeze(1).broadcast_to([_P, 4, 13])
            pt_v = PT[:, 0:52].rearrange("p (c g) -> p c g", g=13)
            ss_v = SS.rearrange("p (c g) -> p c g", g=13)
            nc.vector.tensor_tensor(ss_v, sn_b, pt_v, MULT)

            # ---- gate chain --------------------------------------------
            # g0 = Rx(a0) on |0>: state = (cos, 0, 0, -sin) directly.
            # cos0 is WS col 122+13=135 and -sin0 is WS col 148+39=187, so
            # one strided copy fills comps {0, 3}.
            S = ping.tile([_P, 4], f32, tag="st")
            nc.vector.memset(S[:, 1:3], 0.0)
            nc.vector.tensor_copy(S[:, 0:4:3], WS[:, 135:188:52])
            for g in range(1, 13):
                TMP = ping.tile([_P, 4], f32, tag="tmp")
                if g == 12:
                    SNEW = OUT[:]
                else:
                    SNEW_T = ping.tile([_P, 4], f32, tag="st")
                    SNEW = SNEW_T[:]
                ss_g = SS[:, g:52:13]  # comps [0..3] of gate g, stride 13
                if g % 2 == 0:
                    # Rx: perm = component reversal [s1i, s1r, s0i, s0r]
                    nc.vector.tensor_tensor(TMP[:], S[:][:, ::-1], ss_g, MULT)
                else:
                    # Rz: perm = within-pair swap [s0i, s0r, s1i, s1r]
                    perm = S[:].rearrange("p (a b) -> p a b", b=2)[:, :, ::-1]
                    tmp_v = TMP[:].rearrange("p (a b) -> p a b", b=2)
                    ss_v = ss_g.rearrange("p (a b) -> p a b", b=2)
                    nc.vector.tensor_tensor(tmp_v, perm, ss_v, MULT)
                nc.vector.scalar_tensor_tensor(
                    SNEW, S[:], C[:, g:g + 1], TMP[:], MULT, ADD
                )
                S = SNEW

            SF = OUT[:]
            # ship the state while the expectation is still being computed
            nc.sync.dma_start(out=outp[:], in_=SF)

            # ---- z = s0r^2 + s0i^2 - s1r^2 - s1i^2 per (b, q) lane -----
            SQ = WS[:, 0:4]      # reuse workspace columns
            T2 = WS[:, 4:6]
            Z = WS[:, 6:7]
            nc.vector.tensor_tensor(SQ, SF, SF, MULT)
            nc.vector.tensor_tensor(T2, SQ[:, 0:2], SQ[:, 2:4], SUB)
            nc.vector.tensor_tensor(Z, T2[:, 0:1], T2[:, 1:2], ADD)

            # ---- product over 20 qubit lanes via quadrant rotations ----
            SH = WS[:, 7:8]
            P1 = WS[:, 8:9]
            P2 = WS[:, 9:10]
            P3 = WS[:, 10:11]
            P4 = WS[:, 11:12]
            nc.vector.stream_shuffle(SH, Z, rot_mask(10))
            nc.vector.tensor_tensor(P1, Z, SH, MULT)        # lanes 0..9
            nc.vector.stream_shuffle(SH, P1, rot_mask(5))
            nc.vector.tensor_tensor(P2, P1, SH, MULT)       # lanes 0..4
            nc.vector.stream_shuffle(SH, P2, rot_mask(2))
            nc.vector.tensor_tensor(P3, P2, SH, MULT)       # lanes 0..1
            nc.vector.stream_shuffle(SH, P3, rot_mask(1))
            nc.vector.tensor_tensor(P4, P3, SH, MULT)       # lane 0
            OO = WS[:, 12:13]
            nc.vector.stream_shuffle(SH, P2, rot_mask(4))
            nc.vector.tensor_tensor(OO, P4, SH, MULT)

            # O lives at lanes {0,32,64,96}: 4-element strided read,
            # contiguous 16-byte DRAM write (vs a 128-piece column scatter)
            nc.sync.dma_start(out=o_out[:], in_=WS[0:128:32, 12:13])

    _split_multi_waits(nc)
    _hoist_input_dma(nc)
    _trim_tail_barrier(nc)
    return nc


# CoreSim's race detector requires a full all-engine barrier before the
# sem range-clear; the slimmed tail is HW-safe (every sem's final count
# is explicitly waited on) but sim-rejected, so the sim devloop disables
# it via KERNEL_SLIM_TAIL=0.
_SLIM_TAIL = os.environ.get("KERNEL_SLIM_TAIL", "1") == "1"


def _trim_tail_barrier(nc):
    """Tile's tail emits drain + all-engine barrier, sem range-clear
    (Pool), then a second all-engine barrier.  The second barrier only
    re-syncs engines before the function ends (the NEFF exit handshake
    does that anyway) and the first barrier's only job is ordering the
    range-clear after all sem users — which the explicit completion
    waits already encode.  Keep the SP-side completion waits (output
    visibility at stream end), give the Pool range-clear its own copies
    of those waits, and drop both token-chain barriers."""
    import concourse.mybir as mybir

    bb = nc.m.functions[0].blocks[-1]
    insts = list(bb.instructions)
    cut = None
    for i, ins in enumerate(insts):
        if type(ins).__name__ == "InstISA":
            cut = i
    if cut is None:
        return
    insts = insts[:cut + 1]
    if not _SLIM_TAIL:
        bb.instructions = insts
        return

    # collect the completion waits Tile attached ahead of the SP drain;
    # drop every instruction whose sync touches the barrier sem pair
    def _barrier_sync(si):
        if not si:
            return False
        return any(
            "barrier" in x.ant_name for x in list(si.on_wait) + list(si.on_update)
        )

    waits = []
    keep = []
    pool = []
    for ins in insts:
        tn = type(ins).__name__
        eng = str(ins.engine).split(".")[-1]
        si = getattr(ins, "sync_info", None)
        if _barrier_sync(si):
            continue
        if tn in ("InstNoOp", "InstDrain") and eng == "SP":
            if si:
                waits.extend(si.on_wait)
            keep.append(ins)
        elif tn in ("InstDrain", "InstISA") and eng == "Pool":
            pool.append(ins)

    out = list(keep)
    for k, w in enumerate(waits):
        nop = mybir.InstNoOp(name=f"pool-wait-{k}")
        nop.engine = mybir.EngineType.Pool
        nop.sync_info = mybir.SyncInfo(on_wait=[w], on_update=[])
        nc.register_instruction(nop, overwrite=True)
        out.append(nop)
    out.extend(pool or [])
    bb.instructions = out


def _split_multi_waits(nc, max_waits=1):
    """The walrus build in this toolchain allows at most one embedded sync
    wait per instruction; Tile can emit more (e.g. the kernel-tail drain).
    Hoist excess waits into single-wait NoOps on the same engine queue."""
    import concourse.mybir as mybir

    n = 0
    for bb in nc.m.functions[0].blocks:
        out_list = []
        changed = False
        for ins in bb.instructions:
            si = getattr(ins, "sync_info", None)
            waits = list(si.on_wait) if (si and si.on_wait) else []
            if len(waits) > max_waits:
                for w in waits[:-max_waits]:
                    nop = mybir.InstNoOp(name=f"nop-wait-{n}")
                    n += 1
                    nop.engine = ins.engine
                    nop.sync_info = mybir.SyncInfo(on_wait=[w], on_update=[])
                    nc.register_instruction(nop, overwrite=True)
                    out_list.append(nop)
                ins.sync_info = mybir.SyncInfo(
                    on_wait=waits[-max_waits:], on_update=list(si.on_update)
                )
                changed = True
            out_list.append(ins)
        if changed:
            bb.instructions = out_list


def _hoist_input_dma(nc):
    """Move the (wait-free) input DMA to the front of the program so the
    transfer overlaps the framework preamble barriers instead of queuing
    behind them (~2us saved)."""
    blocks = nc.m.functions[0].blocks
    if len(blocks) < 2:
        return
    tile_bb = blocks[1]
    insts = list(tile_bb.instructions)
    dmas = []
    for ins in insts:
        if type(ins).__name__ == "InstDMACopy":
            si = getattr(ins, "sync_info", None)
            if si and si.on_wait:
                break
            dmas.append(ins)
        if len(dmas) >= 2:
            break
    if not dmas:
        return
    for d in dmas:
        insts.remove(d)
    tile_bb.instructions = insts
    main = list(blocks[0].instructions)
    for i, d in enumerate(dmas):
        main.insert(1 + i, d)
    blocks[0].instructions = main


def _pattern_input():
    """(56,) constant row: per-gate sign patterns in comp-major layout
    [c*13 + g] (cols 0..51) + 4 spare cols."""
    pat = np.empty((13, 4), np.float32)
    for g in range(13):
        pat[g] = (1, -1, 1, -1) if g % 2 == 0 else (1, -1, -1, 1)
    return np.concatenate([pat.T.reshape(-1), np.zeros(4, np.float32)])


def _pack_angles(x, w):
    """(B, Q, 24) raw gate angles in application order."""
    ang = np.empty((_B, _Q, 24), np.float32)
    for i in range(6):
        ang[:, :, 4 * i + 0] = w[i, 0]
        ang[:, :, 4 * i + 1] = w[i, 1]
        ang[:, :, 4 * i + 2] = w[i, 2]
        ang[:, :, 4 * i + 3] = x[:, _COLS[i], :]
    return ang


def _pack_core_input(ang, pat_row, c):
    packed = np.zeros((_P, 81), np.float32)
    for b in range(_BL):
        packed[b * 32:b * 32 + _Q, 0:24] = ang[c * _BL + b]
    packed[:, 25:81] = pat_row  # col 24 stays zero (alpha_6 pad)
    return packed


def kernel(x, weights):
    global LAST_EXEC_TIME_NS, LAST_RESULTS
    from concourse.bass_utils import run_bass_kernel_spmd

    x = np.ascontiguousarray(np.asarray(x, np.float32))
    w = np.ascontiguousarray(np.asarray(weights, np.float32))

    if "nc" not in _CACHE:
        _CACHE["nc"] = _build_nc()
        _CACHE["pat"] = _pattern_input()
    nc = _CACHE["nc"]
    pat_row = _CACHE["pat"]

    ang = _pack_angles(x, w)  # (B, Q, 24)
    in_maps = [
        {"inp": _pack_core_input(ang, pat_row, c)} for c in range(_NCORES)
    ]

    trace = os.environ.get("KERNEL_TRACE", "0") == "1"
    res = run_bass_kernel_spmd(nc, in_maps, list(range(_NCORES)), trace=trace)
    LAST_EXEC_TIME_NS = res.exec_time_ns
    LAST_RESULTS = res

    state = np.empty((_B, _Q, 2), np.complex64)
    O = np.empty((_B, 1, 1), np.complex64)
    for c in range(_NCORES):
        o = np.asarray(res.results[c]["outp"], np.float32)  # (128, 4)
        oo = np.asarray(res.results[c]["o_out"], np.float32).reshape(_BL)
        for b in range(_BL):
            st = o[b * 32:b * 32 + _Q, 0:4].reshape(_Q, 2, 2)
            state[c * _BL + b] = st[..., 0] + 1j * st[..., 1]
            O[c * _BL + b, 0, 0] = np.complex64(oo[b])

    return state.reshape(_B, _Q, 1, 2, 1), O


# revision 43
# speedup vs baseline: 1.0435x; 1.0435x over previous
"""Trainium2 Bass kernel for nn_BasicModel_4054449127788.

Quantum-circuit product-state model: per-(batch, qubit) single-qubit gate
chain (Rx/Rz/Rx + data-encoding Rx, 6 blocks), then Z^(x)n expectation of
the kron-folded wavefunction.

Math used on device: adjacent Rx gates commute and merge (Rx(a)Rx(b) =
Rx(a+b)), collapsing the 24-gate chain to 13 gates.  The Z^(x)n
expectation of a product state factorizes exactly:
    O_b = prod_q (|s_{b,q,0}|^2 - |s_{b,q,1}|^2)
which is numerically *closer* to the reference's f32 kron-fold + signed
sum than an independently-rounded fold replica would be (the fold's own
f32 cancellation noise dominates: ~2e-3 normwise).

Sharding: pure data parallelism — batch 32 split 4-per-core across 8
NeuronCores, no cross-core communication.

Layout on device: 128 partitions = (b_local=4) x (32-lane quadrant), with
qubit q = 0..19 at partition b*32 + q (lanes 20..31 idle).  The state is
4 f32 components [s0r, s0i, s1r, s1i] in the free dim.  Each gate is two
DVE instructions:
    tmp  = perm(S) * (sin * sign_pattern)      (tensor_tensor)
    S'   = (S * cos_perpartition) + tmp        (scalar_tensor_tensor)
where perm is free-dim reversal (Rx) or within-pair swap (Rz).  The
product over the 20 qubits runs in-layout with stream_shuffle quadrant
rotations (no transpose DMA), and O rides in the same output DMA as the
state.
"""

import os
import numpy as np

_B = 32          # full batch
_Q = 20          # qubits
_NCORES = 8
_BL = _B // _NCORES   # batch per core = 4
_P = 128              # partitions: b_local * 32 + q
_COLS = (0, 1, 2, 5, 6, 7)
_HALF_PI = float(np.pi / 2)

_CACHE = {}

# Exposed for test harnesses: exec time of the last traced run (ns).
LAST_EXEC_TIME_NS = None
LAST_RESULTS = None


def _build_nc():
    import concourse.bass as bass
    import concourse.mybir as mybir
    from concourse.tile import TileContext

    f32 = mybir.dt.float32
    ADD = mybir.AluOpType.add
    MULT = mybir.AluOpType.mult
    MIN = mybir.AluOpType.min
    MAX = mybir.AluOpType.max
    SUB = mybir.AluOpType.subtract
    SIN = mybir.ActivationFunctionType.Sin

    nc = bass.Bass("TRN2", target_bir_lowering=False, debug=False)

    # single packed input: cols 0..24 raw gate angles (col 24 zero pad so
    # every merged alpha is a sum of three stride-4 columns), cols 25..80
    # sign patterns
    inp = nc.dram_tensor("inp", [_P, 81], f32, kind="ExternalInput")
    # state output (contiguous) + tiny O output (4 lanes)
    outp = nc.dram_tensor("outp", [_P, 4], f32, kind="ExternalOutput")
    o_out = nc.dram_tensor("o_out", [_BL, 1], f32, kind="ExternalOutput")

    def rot_mask(n):
        return [(i + n) % 32 for i in range(32)]

    with TileContext(nc) as tc:
        with (
            tc.tile_pool(name="cst", bufs=1) as cst,
            tc.tile_pool(name="ping", bufs=2) as ping,
        ):
            # split input DMAs: the small angle block lands first and
            # unblocks DVE prep; the pattern block is only needed later
            # by the SS build
            INA = cst.tile([_P, 25], f32, tag="INA")
            INP = cst.tile([_P, 56], f32, tag="INP")
            nc.sync.dma_start(out=INA[:], in_=inp[:, 0:25])
            nc.sync.dma_start(out=INP[:], in_=inp[:, 25:81])
            A = INA[:]
            PT = INP[:]

            # one workspace tile for all DVE scratch (fewer tile sems ->
            # shorter kernel-tail semaphore-reset sequence)
            WS = cst.tile([_P, 200], f32, tag="WS")
            T6 = WS[:, 0:6]
            T6b = WS[:, 6:12]
            AH = WS[:, 18:44]    # [sin half-angles | cos half-angles]
            U = WS[:, 44:70]
            R = WS[:, 70:96]
            RD = WS[:, 96:122]
            CS = WS[:, 122:148]  # [sin | cos] results
            SS = WS[:, 148:200]  # sin * per-comp sign pattern, comp-major

            OUT = cst.tile([_P, 4], f32, tag="OUT")

            # ---- merge the 24 raw gate angles into 13 half-angles ------
            # gate order g=0..12: Rx(a0), then 6x [Rz(beta_i), Rx(alpha_i)]
            # alpha_i = A[4i-2] + A[4i-1] + A[4i]  (A[24] = 0 pad makes the
            # i=6 group uniform), beta_i = A[4i-3], a0 = A[0].
            PI = float(np.pi)
            MAGIC = float(1.5 * 2.0 ** 23)
            nc.vector.tensor_tensor(T6b[:, :], A[:, 2:23:4], A[:, 3:24:4], ADD)
            nc.vector.tensor_tensor(T6[:, :], T6b[:, :], A[:, 4:25:4], ADD)
            nc.vector.tensor_scalar(AH[:, 0:1], A[:, 0:1], 0.5, None, MULT)
            nc.vector.tensor_scalar(AH[:, 1:12:2], A[:, 1:22:4], 0.5, None, MULT)
            nc.vector.tensor_scalar(AH[:, 2:13:2], T6[:, :], 0.5, None, MULT)

            # ---- cos/sin of half-angles --------------------------------
            # HW Sin needs args in [-pi, pi]; merged angles can exceed it.
            # Range-reduce t -> t - 2pi*round(t/2pi) using the f32
            # magic-constant round (x + 1.5*2^23 - 1.5*2^23), then clamp.
            nc.vector.tensor_scalar(
                AH[:, 13:26], AH[:, 0:13], _HALF_PI, None, ADD
            )
            nc.vector.tensor_scalar(
                U[:, :], AH[:, :], 1.0 / (2.0 * PI), MAGIC, MULT, ADD
            )
            nc.vector.tensor_scalar(R[:, :], U[:, :], MAGIC, None, SUB)
            nc.vector.scalar_tensor_tensor(
                RD[:, :], R[:, :], -2.0 * PI, AH[:, :], MULT, ADD
            )
            nc.vector.tensor_scalar(RD[:, :], RD[:, :], PI, -PI, MIN, MAX)
            nc.scalar.activation(CS[:, :], RD[:, :], SIN, bias=0.0, scale=1.0)
            SN = CS[:, 0:13]
            C = CS[:, 13:26]

            # ---- sin * per-component sign pattern ----------------------
            # comp-major layout SS[p, c*13 + g]; one TT with a broadcast AP
            sn_b = SN.unsqueeze(1).broadcast_to([_P, 4, 13])
            pt_v = PT[:, 0:52].rearrange("p (c g) -> p c g", g=13)
            ss_v = SS.rearrange("p (c g) -> p c g", g=13)
            nc.vector.tensor_tensor(ss_v, sn_b, pt_v, MULT)

            # ---- gate chain --------------------------------------------
            # g0 = Rx(a0) on |0>: state = (cos, 0, 0, -sin) directly.
            # cos0 is WS col 122+13=135 and -sin0 is WS col 148+39=187, so
            # one strided copy fills comps {0, 3}.
            S = ping.tile([_P, 4], f32, tag="st")
            nc.vector.memset(S[:, 1:3], 0.0)
            nc.vector.tensor_copy(S[:, 0:4:3], WS[:, 135:188:52])
            for g in range(1, 13):
                TMP = ping.tile([_P, 4], f32, tag="tmp")
                if g == 12:
                    SNEW = OUT[:]
                else:
                    SNEW_T = ping.tile([_P, 4], f32, tag="st")
                    SNEW = SNEW_T[:]
                ss_g = SS[:, g:52:13]  # comps [0..3] of gate g, stride 13
                if g % 2 == 0:
                    # Rx: perm = component reversal [s1i, s1r, s0i, s0r]
                    nc.vector.tensor_tensor(TMP[:], S[:][:, ::-1], ss_g, MULT)
                else:
                    # Rz: perm = within-pair swap [s0i, s0r, s1i, s1r]
                    perm = S[:].rearrange("p (a b) -> p a b", b=2)[:, :, ::-1]
                    tmp_v = TMP[:].rearrange("p (a b) -> p a b", b=2)
                    ss_v = ss_g.rearrange("p (a b) -> p a b", b=2)
                    nc.vector.tensor_tensor(tmp_v, perm, ss_v, MULT)
                nc.vector.scalar_tensor_tensor(
                    SNEW, S[:], C[:, g:g + 1], TMP[:], MULT, ADD
                )
                S = SNEW

            SF = OUT[:]
            # ship the state while the expectation is still being computed
            nc.sync.dma_start(out=outp[:], in_=SF)

            # ---- z = s0r^2 + s0i^2 - s1r^2 - s1i^2 per (b, q) lane -----
            SQ = WS[:, 0:4]      # reuse workspace columns
            T2 = WS[:, 4:6]
            Z = WS[:, 6:7]
            nc.vector.tensor_tensor(SQ, SF, SF, MULT)
            nc.vector.tensor_tensor(T2, SQ[:, 0:2], SQ[:, 2:4], SUB)
            nc.vector.tensor_tensor(Z, T2[:, 0:1], T2[:, 1:2], ADD)

            # ---- product over 20 qubit lanes via quadrant rotations ----
            SH = WS[:, 7:8]
            P1 = WS[:, 8:9]
            P2 = WS[:, 9:10]
            P3 = WS[:, 10:11]
            P4 = WS[:, 11:12]
            nc.vector.stream_shuffle(SH, Z, rot_mask(10))
            nc.vector.tensor_tensor(P1, Z, SH, MULT)        # lanes 0..9
            nc.vector.stream_shuffle(SH, P1, rot_mask(5))
            nc.vector.tensor_tensor(P2, P1, SH, MULT)       # lanes 0..4
            nc.vector.stream_shuffle(SH, P2, rot_mask(2))
            nc.vector.tensor_tensor(P3, P2, SH, MULT)       # lanes 0..1
            nc.vector.stream_shuffle(SH, P3, rot_mask(1))
            nc.vector.tensor_tensor(P4, P3, SH, MULT)       # lane 0
            OO = WS[:, 12:13]
            nc.vector.stream_shuffle(SH, P2, rot_mask(4))
            nc.vector.tensor_tensor(OO, P4, SH, MULT)

            # O lives at lanes {0,32,64,96}: 4-element strided read,
            # contiguous 16-byte DRAM write (vs a 128-piece column scatter)
            nc.sync.dma_start(out=o_out[:], in_=WS[0:128:32, 12:13])

    _split_multi_waits(nc)
    _hoist_input_dma(nc)
    _trim_tail_barrier(nc)
    return nc


# CoreSim's race detector requires a full all-engine barrier before the
# sem range-clear; the slimmed tail is HW-safe (every sem's final count
# is explicitly waited on) but sim-rejected, so the sim devloop disables
# it via KERNEL_SLIM_TAIL=0.
_SLIM_TAIL = os.environ.get("KERNEL_SLIM_TAIL", "0") == "1"


def _trim_tail_barrier(nc):
    """Tile's tail emits drain + all-engine barrier, sem range-clear
    (Pool), then a second all-engine barrier.  The second barrier only
    re-syncs engines before the function ends (the NEFF exit handshake
    does that anyway) and the first barrier's only job is ordering the
    range-clear after all sem users — which the explicit completion
    waits already encode.  Keep the SP-side completion waits (output
    visibility at stream end), give the Pool range-clear its own copies
    of those waits, and drop both token-chain barriers."""
    import concourse.mybir as mybir

    bb = nc.m.functions[0].blocks[-1]
    insts = list(bb.instructions)
    cut = None
    for i, ins in enumerate(insts):
        if type(ins).__name__ == "InstISA":
            cut = i
    if cut is None:
        return
    insts = insts[:cut + 1]
    if not _SLIM_TAIL:
        bb.instructions = insts
        return

    # collect the completion waits Tile attached ahead of the SP drain;
    # drop every instruction whose sync touches the barrier sem pair
    def _barrier_sync(si):
        if not si:
            return False
        return any(
            "barrier" in x.ant_name for x in list(si.on_wait) + list(si.on_update)
        )

    waits = []
    keep = []
    pool = []
    for ins in insts:
        tn = type(ins).__name__
        eng = str(ins.engine).split(".")[-1]
        si = getattr(ins, "sync_info", None)
        if _barrier_sync(si):
            continue
        if tn in ("InstNoOp", "InstDrain") and eng == "SP":
            if si:
                waits.extend(si.on_wait)
            keep.append(ins)
        elif tn in ("InstDrain", "InstISA") and eng == "Pool":
            pool.append(ins)

    out = list(keep)
    for k, w in enumerate(waits):
        nop = mybir.InstNoOp(name=f"pool-wait-{k}")
        nop.engine = mybir.EngineType.Pool
        nop.sync_info = mybir.SyncInfo(on_wait=[w], on_update=[])
        nc.register_instruction(nop, overwrite=True)
        out.append(nop)
    out.extend(pool or [])
    bb.instructions = out


def _split_multi_waits(nc, max_waits=1):
    """The walrus build in this toolchain allows at most one embedded sync
    wait per instruction; Tile can emit more (e.g. the kernel-tail drain).
    Hoist excess waits into single-wait NoOps on the same engine queue."""
    import concourse.mybir as mybir

    n = 0
    for bb in nc.m.functions[0].blocks:
        out_list = []
        changed = False
        for ins in bb.instructions:
            si = getattr(ins, "sync_info", None)
            waits = list(si.on_wait) if (si and si.on_wait) else []
            if len(waits) > max_waits:
                for w in waits[:-max_waits]:
                    nop = mybir.InstNoOp(name=f"nop-wait-{n}")
                    n += 1
                    nop.engine = ins.engine
                    nop.sync_info = mybir.SyncInfo(on_wait=[w], on_update=[])
                    nc.register_instruction(nop, overwrite=True)
                    out_list.append(nop)
                ins.sync_info = mybir.SyncInfo(
                    on_wait=waits[-max_waits:], on_update=list(si.on_update)
                )
                changed = True
            out_list.append(ins)
        if changed:
            bb.instructions = out_list


def _hoist_input_dma(nc):
    """Move the (wait-free) input DMA to the front of the program so the
    transfer overlaps the framework preamble barriers instead of queuing
    behind them (~2us saved)."""
    blocks = nc.m.functions[0].blocks
    if len(blocks) < 2:
        return
    tile_bb = blocks[1]
    insts = list(tile_bb.instructions)
    dmas = []
    for ins in insts:
        if type(ins).__name__ == "InstDMACopy":
            si = getattr(ins, "sync_info", None)
            if si and si.on_wait:
                break
            dmas.append(ins)
        if len(dmas) >= 2:
            break
    if not dmas:
        return
    for d in dmas:
        insts.remove(d)
    tile_bb.instructions = insts
    main = list(blocks[0].instructions)
    for i, d in enumerate(dmas):
        main.insert(1 + i, d)
    blocks[0].instructions = main


def _pattern_input():
    """(56,) constant row: per-gate sign patterns in comp-major layout
    [c*13 + g] (cols 0..51) + 4 spare cols."""
    pat = np.empty((13, 4), np.float32)
    for g in range(13):
        pat[g] = (1, -1, 1, -1) if g % 2 == 0 else (1, -1, -1, 1)
    return np.concatenate([pat.T.reshape(-1), np.zeros(4, np.float32)])


def _pack_angles(x, w):
    """(B, Q, 24) raw gate angles in application order."""
    ang = np.empty((_B, _Q, 24), np.float32)
    for i in range(6):
        ang[:, :, 4 * i + 0] = w[i, 0]
        ang[:, :, 4 * i + 1] = w[i, 1]
        ang[:, :, 4 * i + 2] = w[i, 2]
        ang[:, :, 4 * i + 3] = x[:, _COLS[i], :]
    return ang


def _pack_core_input(ang, pat_row, c):
    packed = np.zeros((_P, 81), np.float32)
    for b in range(_BL):
        packed[b * 32:b * 32 + _Q, 0:24] = ang[c * _BL + b]
    packed[:, 25:81] = pat_row  # col 24 stays zero (alpha_6 pad)
    return packed


def kernel(x, weights):
    global LAST_EXEC_TIME_NS, LAST_RESULTS
    from concourse.bass_utils import run_bass_kernel_spmd

    x = np.ascontiguousarray(np.asarray(x, np.float32))
    w = np.ascontiguousarray(np.asarray(weights, np.float32))

    if "nc" not in _CACHE:
        _CACHE["nc"] = _build_nc()
        _CACHE["pat"] = _pattern_input()
    nc = _CACHE["nc"]
    pat_row = _CACHE["pat"]

    ang = _pack_angles(x, w)  # (B, Q, 24)
    in_maps = [
        {"inp": _pack_core_input(ang, pat_row, c)} for c in range(_NCORES)
    ]

    trace = os.environ.get("KERNEL_TRACE", "0") == "1"
    res = run_bass_kernel_spmd(nc, in_maps, list(range(_NCORES)), trace=trace)
    LAST_EXEC_TIME_NS = res.exec_time_ns
    LAST_RESULTS = res

    state = np.empty((_B, _Q, 2), np.complex64)
    O = np.empty((_B, 1, 1), np.complex64)
    for c in range(_NCORES):
        o = np.asarray(res.results[c]["outp"], np.float32)  # (128, 4)
        oo = np.asarray(res.results[c]["o_out"], np.float32).reshape(_BL)
        for b in range(_BL):
            st = o[b * 32:b * 32 + _Q, 0:4].reshape(_Q, 2, 2)
            state[c * _BL + b] = st[..., 0] + 1j * st[..., 1]
            O[c * _BL + b, 0, 0] = np.complex64(oo[b])

    return state.reshape(_B, _Q, 1, 2, 1), O


# revision 45
# speedup vs baseline: 1.0449x; 1.0013x over previous
"""Trainium2 Bass kernel for nn_BasicModel_4054449127788.

Quantum-circuit product-state model: per-(batch, qubit) single-qubit gate
chain (Rx/Rz/Rx + data-encoding Rx, 6 blocks), then Z^(x)n expectation of
the kron-folded wavefunction.

Math used on device: adjacent Rx gates commute and merge (Rx(a)Rx(b) =
Rx(a+b)), collapsing the 24-gate chain to 13 gates.  The Z^(x)n
expectation of a product state factorizes exactly:
    O_b = prod_q (|s_{b,q,0}|^2 - |s_{b,q,1}|^2)
which is numerically *closer* to the reference's f32 kron-fold + signed
sum than an independently-rounded fold replica would be (the fold's own
f32 cancellation noise dominates: ~2e-3 normwise).

Sharding: pure data parallelism — batch 32 split 4-per-core across 8
NeuronCores, no cross-core communication.

Layout on device: 128 partitions = (b_local=4) x (32-lane quadrant), with
qubit q = 0..19 at partition b*32 + q (lanes 20..31 idle).  The state is
4 f32 components [s0r, s0i, s1r, s1i] in the free dim.  Each gate is two
DVE instructions:
    tmp  = perm(S) * (sin * sign_pattern)      (tensor_tensor)
    S'   = (S * cos_perpartition) + tmp        (scalar_tensor_tensor)
where perm is free-dim reversal (Rx) or within-pair swap (Rz).  The
product over the 20 qubits runs in-layout with stream_shuffle quadrant
rotations (no transpose DMA); the state ships in its own contiguous DMA
as soon as the last gate lands, and only a 16-byte O write rides the
tail.  Post-passes split multi-wait sync (walrus limit), hoist the
input DMAs ahead of the preamble, and drop Tile's redundant second
tail barrier.
"""

import os
import numpy as np

_B = 32          # full batch
_Q = 20          # qubits
_NCORES = 8
_BL = _B // _NCORES   # batch per core = 4
_P = 128              # partitions: b_local * 32 + q
_COLS = (0, 1, 2, 5, 6, 7)
_HALF_PI = float(np.pi / 2)

_CACHE = {}

# Exposed for test harnesses: exec time of the last traced run (ns).
LAST_EXEC_TIME_NS = None
LAST_RESULTS = None


def _build_nc():
    import concourse.bass as bass
    import concourse.mybir as mybir
    from concourse.tile import TileContext

    f32 = mybir.dt.float32
    ADD = mybir.AluOpType.add
    MULT = mybir.AluOpType.mult
    MIN = mybir.AluOpType.min
    MAX = mybir.AluOpType.max
    SUB = mybir.AluOpType.subtract
    SIN = mybir.ActivationFunctionType.Sin

    nc = bass.Bass("TRN2", target_bir_lowering=False, debug=False)

    # single packed input: cols 0..24 raw gate angles (col 24 zero pad so
    # every merged alpha is a sum of three stride-4 columns), cols 25..80
    # sign patterns
    inp = nc.dram_tensor("inp", [_P, 81], f32, kind="ExternalInput")
    # state output (contiguous) + tiny O output (4 lanes)
    outp = nc.dram_tensor("outp", [_P, 4], f32, kind="ExternalOutput")
    o_out = nc.dram_tensor("o_out", [_BL, 1], f32, kind="ExternalOutput")

    def rot_mask(n):
        return [(i + n) % 32 for i in range(32)]

    with TileContext(nc) as tc:
        with (
            tc.tile_pool(name="cst", bufs=1) as cst,
            tc.tile_pool(name="ping", bufs=2) as ping,
        ):
            # split input DMAs: the small angle block lands first and
            # unblocks DVE prep; the pattern block is only needed later
            # by the SS build
            INA = cst.tile([_P, 25], f32, tag="INA")
            INP = cst.tile([_P, 56], f32, tag="INP")
            nc.sync.dma_start(out=INA[:], in_=inp[:, 0:25])
            nc.sync.dma_start(out=INP[:], in_=inp[:, 25:81])
            A = INA[:]
            PT = INP[:]

            # one workspace tile for all DVE scratch (fewer tile sems ->
            # shorter kernel-tail semaphore-reset sequence)
            WS = cst.tile([_P, 200], f32, tag="WS")
            T6 = WS[:, 0:6]
            T6b = WS[:, 6:12]
            AH = WS[:, 18:44]    # [sin half-angles | cos half-angles]
            U = WS[:, 44:70]
            R = WS[:, 70:96]
            RD = WS[:, 96:122]
            CS = WS[:, 122:148]  # [sin | cos] results
            SS = WS[:, 148:200]  # sin * per-comp sign pattern, comp-major

            OUT = cst.tile([_P, 4], f32, tag="OUT")

            # ---- merge the 24 raw gate angles into 13 half-angles ------
            # gate order g=0..12: Rx(a0), then 6x [Rz(beta_i), Rx(alpha_i)]
            # alpha_i = A[4i-2] + A[4i-1] + A[4i]  (A[24] = 0 pad makes the
            # i=6 group uniform), beta_i = A[4i-3], a0 = A[0].
            PI = float(np.pi)
            MAGIC = float(1.5 * 2.0 ** 23)
            nc.vector.tensor_tensor(T6b[:, :], A[:, 2:23:4], A[:, 3:24:4], ADD)
            nc.vector.tensor_tensor(T6[:, :], T6b[:, :], A[:, 4:25:4], ADD)
            nc.vector.tensor_scalar(AH[:, 0:1], A[:, 0:1], 0.5, None, MULT)
            nc.vector.tensor_scalar(AH[:, 1:12:2], A[:, 1:22:4], 0.5, None, MULT)
            nc.vector.tensor_scalar(AH[:, 2:13:2], T6[:, :], 0.5, None, MULT)

            # ---- cos/sin of half-angles --------------------------------
            # HW Sin needs args in [-pi, pi]; merged angles can exceed it.
            # Range-reduce t -> t - 2pi*round(t/2pi) using the f32
            # magic-constant round (x + 1.5*2^23 - 1.5*2^23), then clamp.
            nc.vector.tensor_scalar(
                AH[:, 13:26], AH[:, 0:13], _HALF_PI, None, ADD
            )
            nc.vector.tensor_scalar(
                U[:, :], AH[:, :], 1.0 / (2.0 * PI), MAGIC, MULT, ADD
            )
            nc.vector.tensor_scalar(R[:, :], U[:, :], MAGIC, None, SUB)
            nc.vector.scalar_tensor_tensor(
                RD[:, :], R[:, :], -2.0 * PI, AH[:, :], MULT, ADD
            )
            nc.vector.tensor_scalar(RD[:, :], RD[:, :], PI, -PI, MIN, MAX)
            # split Sin: cols 0..13 (all sines + cos0) unblock the SS
            # build and state init; the remaining cosines overlap them on
            # the Scalar engine and land before gate 1 consumes them
            nc.scalar.activation(CS[:, 0:14], RD[:, 0:14], SIN, bias=0.0, scale=1.0)
            nc.scalar.activation(CS[:, 14:26], RD[:, 14:26], SIN, bias=0.0, scale=1.0)
            SN = CS[:, 0:13]
            C = CS[:, 13:26]

            # ---- sin * per-component sign pattern ----------------------
            # comp-major layout SS[p, c*13 + g]; one TT with a broadcast AP
            sn_b = SN.unsqueeze(1).broadcast_to([_P, 4, 13])
            pt_v = PT[:, 0:52].rearrange("p (c g) -> p c g", g=13)
            ss_v = SS.rearrange("p (c g) -> p c g", g=13)
            nc.vector.tensor_tensor(ss_v, sn_b, pt_v, MULT)

            # ---- gate chain --------------------------------------------
            # g0 = Rx(a0) on |0>: state = (cos, 0, 0, -sin) directly.
            # cos0 is WS col 122+13=135 and -sin0 is WS col 148+39=187, so
            # one strided copy fills comps {0, 3}.
            S = ping.tile([_P, 4], f32, tag="st")
            nc.vector.memset(S[:, 1:3], 0.0)
            nc.vector.tensor_copy(S[:, 0:4:3], WS[:, 135:188:52])
            for g in range(1, 13):
                TMP = ping.tile([_P, 4], f32, tag="tmp")
                if g == 12:
                    SNEW = OUT[:]
                else:
                    SNEW_T = ping.tile([_P, 4], f32, tag="st")
                    SNEW = SNEW_T[:]
                ss_g = SS[:, g:52:13]  # comps [0..3] of gate g, stride 13
                if g % 2 == 0:
                    # Rx: perm = component reversal [s1i, s1r, s0i, s0r]
                    nc.vector.tensor_tensor(TMP[:], S[:][:, ::-1], ss_g, MULT)
                else:
                    # Rz: perm = within-pair swap [s0i, s0r, s1i, s1r]
                    perm = S[:].rearrange("p (a b) -> p a b", b=2)[:, :, ::-1]
                    tmp_v = TMP[:].rearrange("p (a b) -> p a b", b=2)
                    ss_v = ss_g.rearrange("p (a b) -> p a b", b=2)
                    nc.vector.tensor_tensor(tmp_v, perm, ss_v, MULT)
                nc.vector.scalar_tensor_tensor(
                    SNEW, S[:], C[:, g:g + 1], TMP[:], MULT, ADD
                )
                S = SNEW

            SF = OUT[:]
            # ship the state while the expectation is still being computed
            nc.sync.dma_start(out=outp[:], in_=SF)

            # ---- z = s0r^2 + s0i^2 - s1r^2 - s1i^2 per (b, q) lane -----
            SQ = WS[:, 0:4]      # reuse workspace columns
            T2 = WS[:, 4:6]
            Z = WS[:, 6:7]
            nc.vector.tensor_tensor(SQ, SF, SF, MULT)
            nc.vector.tensor_tensor(T2, SQ[:, 0:2], SQ[:, 2:4], SUB)
            nc.vector.tensor_tensor(Z, T2[:, 0:1], T2[:, 1:2], ADD)

            # ---- product over 20 qubit lanes via quadrant rotations ----
            SH = WS[:, 7:8]
            P1 = WS[:, 8:9]
            P2 = WS[:, 9:10]
            P3 = WS[:, 10:11]
            P4 = WS[:, 11:12]
            nc.vector.stream_shuffle(SH, Z, rot_mask(10))
            nc.vector.tensor_tensor(P1, Z, SH, MULT)        # lanes 0..9
            nc.vector.stream_shuffle(SH, P1, rot_mask(5))
            nc.vector.tensor_tensor(P2, P1, SH, MULT)       # lanes 0..4
            nc.vector.stream_shuffle(SH, P2, rot_mask(2))
            nc.vector.tensor_tensor(P3, P2, SH, MULT)       # lanes 0..1
            nc.vector.stream_shuffle(SH, P3, rot_mask(1))
            nc.vector.tensor_tensor(P4, P3, SH, MULT)       # lane 0
            OO = WS[:, 12:13]
            nc.vector.stream_shuffle(SH, P2, rot_mask(4))
            nc.vector.tensor_tensor(OO, P4, SH, MULT)

            # O lives at lanes {0,32,64,96}: 4-element strided read,
            # contiguous 16-byte DRAM write (vs a 128-piece column scatter)
            nc.sync.dma_start(out=o_out[:], in_=WS[0:128:32, 12:13])

    _split_multi_waits(nc)
    _hoist_input_dma(nc)
    _trim_tail_barrier(nc)
    return nc


# CoreSim's race detector requires a full all-engine barrier before the
# sem range-clear; the slimmed tail is HW-safe (every sem's final count
# is explicitly waited on) but sim-rejected, so the sim devloop disables
# it via KERNEL_SLIM_TAIL=0.
_SLIM_TAIL = os.environ.get("KERNEL_SLIM_TAIL", "0") == "1"


def _trim_tail_barrier(nc):
    """Tile's tail emits drain + all-engine barrier, sem range-clear
    (Pool), then a second all-engine barrier.  The second barrier only
    re-syncs engines before the function ends (the NEFF exit handshake
    does that anyway) and the first barrier's only job is ordering the
    range-clear after all sem users — which the explicit completion
    waits already encode.  Keep the SP-side completion waits (output
    visibility at stream end), give the Pool range-clear its own copies
    of those waits, and drop both token-chain barriers."""
    import concourse.mybir as mybir

    bb = nc.m.functions[0].blocks[-1]
    insts = list(bb.instructions)
    cut = None
    for i, ins in enumerate(insts):
        if type(ins).__name__ == "InstISA":
            cut = i
    if cut is None:
        return
    insts = insts[:cut + 1]
    if not _SLIM_TAIL:
        bb.instructions = insts
        return

    # collect the completion waits Tile attached ahead of the SP drain;
    # drop every instruction whose sync touches the barrier sem pair
    def _barrier_sync(si):
        if not si:
            return False
        return any(
            "barrier" in x.ant_name for x in list(si.on_wait) + list(si.on_update)
        )

    waits = []
    keep = []
    pool = []
    for ins in insts:
        tn = type(ins).__name__
        eng = str(ins.engine).split(".")[-1]
        si = getattr(ins, "sync_info", None)
        if _barrier_sync(si):
            continue
        if tn in ("InstNoOp", "InstDrain") and eng == "SP":
            if si:
                waits.extend(si.on_wait)
            keep.append(ins)
        elif tn in ("InstDrain", "InstISA") and eng == "Pool":
            pool.append(ins)

    out = list(keep)
    for k, w in enumerate(waits):
        nop = mybir.InstNoOp(name=f"pool-wait-{k}")
        nop.engine = mybir.EngineType.Pool
        nop.sync_info = mybir.SyncInfo(on_wait=[w], on_update=[])
        nc.register_instruction(nop, overwrite=True)
        out.append(nop)
    out.extend(pool or [])
    bb.instructions = out


def _split_multi_waits(nc, max_waits=1):
    """The walrus build in this toolchain allows at most one embedded sync
    wait per instruction; Tile can emit more (e.g. the kernel-tail drain).
    Hoist excess waits into single-wait NoOps on the same engine queue."""
    import concourse.mybir as mybir

    n = 0
    for bb in nc.m.functions[0].blocks:
        out_list = []
        changed = False
        for ins in bb.instructions:
            si = getattr(ins, "sync_info", None)
            waits = list(si.on_wait) if (si and si.on_wait) else []
            if len(waits) > max_waits:
                for w in waits[:-max_waits]:
                    nop = mybir.InstNoOp(name=f"nop-wait-{n}")
                    n += 1
                    nop.engine = ins.engine
                    nop.sync_info = mybir.SyncInfo(on_wait=[w], on_update=[])
                    nc.register_instruction(nop, overwrite=True)
                    out_list.append(nop)
                ins.sync_info = mybir.SyncInfo(
                    on_wait=waits[-max_waits:], on_update=list(si.on_update)
                )
                changed = True
            out_list.append(ins)
        if changed:
            bb.instructions = out_list


def _hoist_input_dma(nc):
    """Move the (wait-free) input DMA to the front of the program so the
    transfer overlaps the framework preamble barriers instead of queuing
    behind them (~2us saved)."""
    blocks = nc.m.functions[0].blocks
    if len(blocks) < 2:
        return
    tile_bb = blocks[1]
    insts = list(tile_bb.instructions)
    dmas = []
    for ins in insts:
        if type(ins).__name__ == "InstDMACopy":
            si = getattr(ins, "sync_info", None)
            if si and si.on_wait:
                break
            dmas.append(ins)
        if len(dmas) >= 2:
            break
    if not dmas:
        return
    for d in dmas:
        insts.remove(d)
    tile_bb.instructions = insts
    main = list(blocks[0].instructions)
    for i, d in enumerate(dmas):
        main.insert(1 + i, d)
    blocks[0].instructions = main


def _pattern_input():
    """(56,) constant row: per-gate sign patterns in comp-major layout
    [c*13 + g] (cols 0..51) + 4 spare cols."""
    pat = np.empty((13, 4), np.float32)
    for g in range(13):
        pat[g] = (1, -1, 1, -1) if g % 2 == 0 else (1, -1, -1, 1)
    return np.concatenate([pat.T.reshape(-1), np.zeros(4, np.float32)])


def _pack_angles(x, w):
    """(B, Q, 24) raw gate angles in application order."""
    ang = np.empty((_B, _Q, 24), np.float32)
    for i in range(6):
        ang[:, :, 4 * i + 0] = w[i, 0]
        ang[:, :, 4 * i + 1] = w[i, 1]
        ang[:, :, 4 * i + 2] = w[i, 2]
        ang[:, :, 4 * i + 3] = x[:, _COLS[i], :]
    return ang


def _pack_core_input(ang, pat_row, c):
    packed = np.zeros((_P, 81), np.float32)
    for b in range(_BL):
        packed[b * 32:b * 32 + _Q, 0:24] = ang[c * _BL + b]
    packed[:, 25:81] = pat_row  # col 24 stays zero (alpha_6 pad)
    return packed


def kernel(x, weights):
    global LAST_EXEC_TIME_NS, LAST_RESULTS
    from concourse.bass_utils import run_bass_kernel_spmd

    x = np.ascontiguousarray(np.asarray(x, np.float32))
    w = np.ascontiguousarray(np.asarray(weights, np.float32))

    if "nc" not in _CACHE:
        _CACHE["nc"] = _build_nc()
        _CACHE["pat"] = _pattern_input()
    nc = _CACHE["nc"]
    pat_row = _CACHE["pat"]

    ang = _pack_angles(x, w)  # (B, Q, 24)
    in_maps = [
        {"inp": _pack_core_input(ang, pat_row, c)} for c in range(_NCORES)
    ]

    trace = os.environ.get("KERNEL_TRACE", "0") == "1"
    res = run_bass_kernel_spmd(nc, in_maps, list(range(_NCORES)), trace=trace)
    LAST_EXEC_TIME_NS = res.exec_time_ns
    LAST_RESULTS = res

    state = np.empty((_B, _Q, 2), np.complex64)
    O = np.empty((_B, 1, 1), np.complex64)
    for c in range(_NCORES):
        o = np.asarray(res.results[c]["outp"], np.float32)  # (128, 4)
        oo = np.asarray(res.results[c]["o_out"], np.float32).reshape(_BL)
        for b in range(_BL):
            st = o[b * 32:b * 32 + _Q, 0:4].reshape(_Q, 2, 2)
            state[c * _BL + b] = st[..., 0] + 1j * st[..., 1]
            O[c * _BL + b, 0, 0] = np.complex64(oo[b])

    return state.reshape(_B, _Q, 1, 2, 1), O


# revision 46
# speedup vs baseline: 1.0588x; 1.0134x over previous
"""Trainium2 Bass kernel for nn_BasicModel_4054449127788.

Quantum-circuit product-state model: per-(batch, qubit) single-qubit gate
chain (Rx/Rz/Rx + data-encoding Rx, 6 blocks), then Z^(x)n expectation of
the kron-folded wavefunction.

Math used on device: adjacent Rx gates commute and merge (Rx(a)Rx(b) =
Rx(a+b)), collapsing the 24-gate chain to 13 gates.  The Z^(x)n
expectation of a product state factorizes exactly:
    O_b = prod_q (|s_{b,q,0}|^2 - |s_{b,q,1}|^2)
which is numerically *closer* to the reference's f32 kron-fold + signed
sum than an independently-rounded fold replica would be (the fold's own
f32 cancellation noise dominates: ~2e-3 normwise).

Sharding: pure data parallelism — batch 32 split 4-per-core across 8
NeuronCores, no cross-core communication.

Layout on device: 128 partitions = (b_local=4) x (32-lane quadrant), with
qubit q = 0..19 at partition b*32 + q (lanes 20..31 idle).  The state is
4 f32 components [s0r, s0i, s1r, s1i] in the free dim.  Each gate is two
DVE instructions:
    tmp  = perm(S) * (sin * sign_pattern)      (tensor_tensor)
    S'   = (S * cos_perpartition) + tmp        (scalar_tensor_tensor)
where perm is free-dim reversal (Rx) or within-pair swap (Rz).  The
product over the 20 qubits runs in-layout with stream_shuffle quadrant
rotations (no transpose DMA); the state ships in its own contiguous DMA
as soon as the last gate lands, and only a 16-byte O write rides the
tail.  Post-passes split multi-wait sync (walrus limit), hoist the
input DMAs ahead of the preamble, and drop Tile's redundant second
tail barrier.
"""

import os
import numpy as np

_B = 32          # full batch
_Q = 20          # qubits
_NCORES = 8
_BL = _B // _NCORES   # batch per core = 4
_P = 128              # partitions: b_local * 32 + q
_COLS = (0, 1, 2, 5, 6, 7)
_HALF_PI = float(np.pi / 2)

_CACHE = {}

# Exposed for test harnesses: exec time of the last traced run (ns).
LAST_EXEC_TIME_NS = None
LAST_RESULTS = None


def _build_nc():
    import concourse.bass as bass
    import concourse.mybir as mybir
    from concourse.tile import TileContext

    f32 = mybir.dt.float32
    ADD = mybir.AluOpType.add
    MULT = mybir.AluOpType.mult
    MIN = mybir.AluOpType.min
    MAX = mybir.AluOpType.max
    SUB = mybir.AluOpType.subtract
    SIN = mybir.ActivationFunctionType.Sin

    nc = bass.Bass("TRN2", target_bir_lowering=False, debug=False)

    # single packed input: cols 0..24 raw gate angles (col 24 zero pad so
    # every merged alpha is a sum of three stride-4 columns), cols 25..80
    # sign patterns
    inp = nc.dram_tensor("inp", [_P, 81], f32, kind="ExternalInput")
    # state output (contiguous) + tiny O output (4 lanes)
    outp = nc.dram_tensor("outp", [_P, 4], f32, kind="ExternalOutput")
    o_out = nc.dram_tensor("o_out", [_BL, 1], f32, kind="ExternalOutput")

    def rot_mask(n):
        return [(i + n) % 32 for i in range(32)]

    with TileContext(nc) as tc:
        with (
            tc.tile_pool(name="cst", bufs=1) as cst,
            tc.tile_pool(name="ping", bufs=2) as ping,
        ):
            # split input DMAs: the small angle block lands first and
            # unblocks DVE prep; the pattern block is only needed later
            # by the SS build
            INA = cst.tile([_P, 25], f32, tag="INA")
            INP = cst.tile([_P, 56], f32, tag="INP")
            nc.sync.dma_start(out=INA[:], in_=inp[:, 0:25])
            nc.sync.dma_start(out=INP[:], in_=inp[:, 25:81])
            A = INA[:]
            PT = INP[:]

            # one workspace tile for all DVE scratch (fewer tile sems ->
            # shorter kernel-tail semaphore-reset sequence)
            WS = cst.tile([_P, 200], f32, tag="WS")
            T6 = WS[:, 0:6]
            T6b = WS[:, 6:12]
            AH = WS[:, 18:44]    # [sin half-angles | cos half-angles]
            U = WS[:, 44:70]
            R = WS[:, 70:96]
            RD = WS[:, 96:122]
            CS = WS[:, 122:148]  # [sin | cos] results
            SS = WS[:, 148:200]  # sin * per-comp sign pattern, comp-major

            OUT = cst.tile([_P, 4], f32, tag="OUT")

            # ---- merge the 24 raw gate angles into 13 half-angles ------
            # gate order g=0..12: Rx(a0), then 6x [Rz(beta_i), Rx(alpha_i)]
            # alpha_i = A[4i-2] + A[4i-1] + A[4i]  (A[24] = 0 pad makes the
            # i=6 group uniform), beta_i = A[4i-3], a0 = A[0].
            PI = float(np.pi)
            MAGIC = float(1.5 * 2.0 ** 23)
            nc.vector.tensor_tensor(T6b[:, :], A[:, 2:23:4], A[:, 3:24:4], ADD)
            nc.vector.tensor_tensor(T6[:, :], T6b[:, :], A[:, 4:25:4], ADD)
            nc.vector.tensor_scalar(AH[:, 0:1], A[:, 0:1], 0.5, None, MULT)
            nc.vector.tensor_scalar(AH[:, 1:12:2], A[:, 1:22:4], 0.5, None, MULT)
            nc.vector.tensor_scalar(AH[:, 2:13:2], T6[:, :], 0.5, None, MULT)

            # ---- cos/sin of half-angles --------------------------------
            # HW Sin needs args in [-pi, pi]; merged angles can exceed it.
            # Range-reduce t -> t - 2pi*round(t/2pi) using the f32
            # magic-constant round (x + 1.5*2^23 - 1.5*2^23), then clamp.
            nc.vector.tensor_scalar(
                AH[:, 13:26], AH[:, 0:13], _HALF_PI, None, ADD
            )
            nc.vector.tensor_scalar(
                U[:, :], AH[:, :], 1.0 / (2.0 * PI), MAGIC, MULT, ADD
            )
            nc.vector.tensor_scalar(R[:, :], U[:, :], MAGIC, None, SUB)
            nc.vector.scalar_tensor_tensor(
                RD[:, :], R[:, :], -2.0 * PI, AH[:, :], MULT, ADD
            )
            nc.vector.tensor_scalar(RD[:, :], RD[:, :], PI, -PI, MIN, MAX)
            # split Sin: cols 0..13 (all sines + cos0) unblock the SS
            # build and state init; the remaining cosines overlap them on
            # the Scalar engine and land before gate 1 consumes them
            nc.scalar.activation(CS[:, 0:14], RD[:, 0:14], SIN, bias=0.0, scale=1.0)
            nc.scalar.activation(CS[:, 14:26], RD[:, 14:26], SIN, bias=0.0, scale=1.0)
            SN = CS[:, 0:13]
            C = CS[:, 13:26]

            # ---- sin * per-component sign pattern ----------------------
            # comp-major layout SS[p, c*13 + g]; one TT with a broadcast AP
            sn_b = SN.unsqueeze(1).broadcast_to([_P, 4, 13])
            pt_v = PT[:, 0:52].rearrange("p (c g) -> p c g", g=13)
            ss_v = SS.rearrange("p (c g) -> p c g", g=13)
            nc.vector.tensor_tensor(ss_v, sn_b, pt_v, MULT)

            # ---- gate chain --------------------------------------------
            # g0 = Rx(a0) on |0>: state = (cos, 0, 0, -sin) directly.
            # cos0 is WS col 122+13=135 and -sin0 is WS col 148+39=187, so
            # one strided copy fills comps {0, 3}.
            S = ping.tile([_P, 4], f32, tag="st")
            nc.vector.memset(S[:, 1:3], 0.0)
            nc.vector.tensor_copy(S[:, 0:4:3], WS[:, 135:188:52])
            for g in range(1, 13):
                TMP = ping.tile([_P, 4], f32, tag="tmp")
                if g == 12:
                    SNEW = OUT[:]
                else:
                    SNEW_T = ping.tile([_P, 4], f32, tag="st")
                    SNEW = SNEW_T[:]
                ss_g = SS[:, g:52:13]  # comps [0..3] of gate g, stride 13
                if g % 2 == 0:
                    # Rx: perm = component reversal [s1i, s1r, s0i, s0r]
                    nc.vector.tensor_tensor(TMP[:], S[:][:, ::-1], ss_g, MULT)
                else:
                    # Rz: perm = within-pair swap [s0i, s0r, s1i, s1r]
                    perm = S[:].rearrange("p (a b) -> p a b", b=2)[:, :, ::-1]
                    tmp_v = TMP[:].rearrange("p (a b) -> p a b", b=2)
                    ss_v = ss_g.rearrange("p (a b) -> p a b", b=2)
                    nc.vector.tensor_tensor(tmp_v, perm, ss_v, MULT)
                nc.vector.scalar_tensor_tensor(
                    SNEW, S[:], C[:, g:g + 1], TMP[:], MULT, ADD
                )
                S = SNEW

            SF = OUT[:]
            # ship the state while the expectation is still being computed
            nc.sync.dma_start(out=outp[:], in_=SF)

            # ---- z = s0r^2 + s0i^2 - s1r^2 - s1i^2 per (b, q) lane -----
            # fused: out = SQ[0:2] - SQ[2:4], accum_out sums it into z
            SQ = WS[:, 0:4]      # reuse workspace columns
            T2 = WS[:, 4:6]
            Z = WS[:, 6:7]
            nc.vector.tensor_tensor(SQ, SF, SF, MULT)
            nc.vector.scalar_tensor_tensor(
                T2, SQ[:, 0:2], 1.0, SQ[:, 2:4], MULT, SUB, accum_out=Z
            )

            # ---- product over 20 qubit lanes via quadrant rotations ----
            SH = WS[:, 7:8]
            P1 = WS[:, 8:9]
            P2 = WS[:, 9:10]
            P3 = WS[:, 10:11]
            P4 = WS[:, 11:12]
            nc.vector.stream_shuffle(SH, Z, rot_mask(10))
            nc.vector.tensor_tensor(P1, Z, SH, MULT)        # lanes 0..9
            nc.vector.stream_shuffle(SH, P1, rot_mask(5))
            nc.vector.tensor_tensor(P2, P1, SH, MULT)       # lanes 0..4
            nc.vector.stream_shuffle(SH, P2, rot_mask(2))
            nc.vector.tensor_tensor(P3, P2, SH, MULT)       # lanes 0..1
            nc.vector.stream_shuffle(SH, P3, rot_mask(1))
            nc.vector.tensor_tensor(P4, P3, SH, MULT)       # lane 0
            OO = WS[:, 12:13]
            nc.vector.stream_shuffle(SH, P2, rot_mask(4))
            nc.vector.tensor_tensor(OO, P4, SH, MULT)

            # O lives at lanes {0,32,64,96}: 4-element strided read,
            # contiguous 16-byte DRAM write (vs a 128-piece column scatter)
            nc.sync.dma_start(out=o_out[:], in_=WS[0:128:32, 12:13])

    _split_multi_waits(nc)
    _hoist_input_dma(nc)
    _trim_tail_barrier(nc)
    return nc


# CoreSim's race detector requires a full all-engine barrier before the
# sem range-clear; the slimmed tail is HW-safe (every sem's final count
# is explicitly waited on) but sim-rejected, so the sim devloop disables
# it via KERNEL_SLIM_TAIL=0.
_SLIM_TAIL = os.environ.get("KERNEL_SLIM_TAIL", "0") == "1"


def _trim_tail_barrier(nc):
    """Tile's tail emits drain + all-engine barrier, sem range-clear
    (Pool), then a second all-engine barrier.  The second barrier only
    re-syncs engines before the function ends (the NEFF exit handshake
    does that anyway) and the first barrier's only job is ordering the
    range-clear after all sem users — which the explicit completion
    waits already encode.  Keep the SP-side completion waits (output
    visibility at stream end), give the Pool range-clear its own copies
    of those waits, and drop both token-chain barriers."""
    import concourse.mybir as mybir

    bb = nc.m.functions[0].blocks[-1]
    insts = list(bb.instructions)
    cut = None
    for i, ins in enumerate(insts):
        if type(ins).__name__ == "InstISA":
            cut = i
    if cut is None:
        return
    insts = insts[:cut + 1]
    if not _SLIM_TAIL:
        bb.instructions = insts
        return

    # collect the completion waits Tile attached ahead of the SP drain;
    # drop every instruction whose sync touches the barrier sem pair
    def _barrier_sync(si):
        if not si:
            return False
        return any(
            "barrier" in x.ant_name for x in list(si.on_wait) + list(si.on_update)
        )

    waits = []
    keep = []
    pool = []
    for ins in insts:
        tn = type(ins).__name__
        eng = str(ins.engine).split(".")[-1]
        si = getattr(ins, "sync_info", None)
        if _barrier_sync(si):
            continue
        if tn in ("InstNoOp", "InstDrain") and eng == "SP":
            if si:
                waits.extend(si.on_wait)
            keep.append(ins)
        elif tn in ("InstDrain", "InstISA") and eng == "Pool":
            pool.append(ins)

    out = list(keep)
    for k, w in enumerate(waits):
        nop = mybir.InstNoOp(name=f"pool-wait-{k}")
        nop.engine = mybir.EngineType.Pool
        nop.sync_info = mybir.SyncInfo(on_wait=[w], on_update=[])
        nc.register_instruction(nop, overwrite=True)
        out.append(nop)
    out.extend(pool or [])
    bb.instructions = out


def _split_multi_waits(nc, max_waits=1):
    """The walrus build in this toolchain allows at most one embedded sync
    wait per instruction; Tile can emit more (e.g. the kernel-tail drain).
    Hoist excess waits into single-wait NoOps on the same engine queue."""
    import concourse.mybir as mybir

    n = 0
    for bb in nc.m.functions[0].blocks:
        out_list = []
        changed = False
        for ins in bb.instructions:
            si = getattr(ins, "sync_info", None)
            waits = list(si.on_wait) if (si and si.on_wait) else []
            if len(waits) > max_waits:
                for w in waits[:-max_waits]:
                    nop = mybir.InstNoOp(name=f"nop-wait-{n}")
                    n += 1
                    nop.engine = ins.engine
                    nop.sync_info = mybir.SyncInfo(on_wait=[w], on_update=[])
                    nc.register_instruction(nop, overwrite=True)
                    out_list.append(nop)
                ins.sync_info = mybir.SyncInfo(
                    on_wait=waits[-max_waits:], on_update=list(si.on_update)
                )
                changed = True
            out_list.append(ins)
        if changed:
            bb.instructions = out_list


def _hoist_input_dma(nc):
    """Move the (wait-free) input DMA to the front of the program so the
    transfer overlaps the framework preamble barriers instead of queuing
    behind them (~2us saved)."""
    blocks = nc.m.functions[0].blocks
    if len(blocks) < 2:
        return
    tile_bb = blocks[1]
    insts = list(tile_bb.instructions)
    dmas = []
    for ins in insts:
        if type(ins).__name__ == "InstDMACopy":
            si = getattr(ins, "sync_info", None)
            if si and si.on_wait:
                break
            dmas.append(ins)
        if len(dmas) >= 2:
            break
    if not dmas:
        return
    for d in dmas:
        insts.remove(d)
    tile_bb.instructions = insts
    main = list(blocks[0].instructions)
    for i, d in enumerate(dmas):
        main.insert(1 + i, d)
    blocks[0].instructions = main


def _pattern_input():
    """(56,) constant row: per-gate sign patterns in comp-major layout
    [c*13 + g] (cols 0..51) + 4 spare cols."""
    pat = np.empty((13, 4), np.float32)
    for g in range(13):
        pat[g] = (1, -1, 1, -1) if g % 2 == 0 else (1, -1, -1, 1)
    return np.concatenate([pat.T.reshape(-1), np.zeros(4, np.float32)])


def _pack_angles(x, w):
    """(B, Q, 24) raw gate angles in application order."""
    ang = np.empty((_B, _Q, 24), np.float32)
    for i in range(6):
        ang[:, :, 4 * i + 0] = w[i, 0]
        ang[:, :, 4 * i + 1] = w[i, 1]
        ang[:, :, 4 * i + 2] = w[i, 2]
        ang[:, :, 4 * i + 3] = x[:, _COLS[i], :]
    return ang


def _pack_core_input(ang, pat_row, c):
    packed = np.zeros((_P, 81), np.float32)
    for b in range(_BL):
        packed[b * 32:b * 32 + _Q, 0:24] = ang[c * _BL + b]
    packed[:, 25:81] = pat_row  # col 24 stays zero (alpha_6 pad)
    return packed


def kernel(x, weights):
    global LAST_EXEC_TIME_NS, LAST_RESULTS
    from concourse.bass_utils import run_bass_kernel_spmd

    x = np.ascontiguousarray(np.asarray(x, np.float32))
    w = np.ascontiguousarray(np.asarray(weights, np.float32))

    if "nc" not in _CACHE:
        _CACHE["nc"] = _build_nc()
        _CACHE["pat"] = _pattern_input()
    nc = _CACHE["nc"]
    pat_row = _CACHE["pat"]

    ang = _pack_angles(x, w)  # (B, Q, 24)
    in_maps = [
        {"inp": _pack_core_input(ang, pat_row, c)} for c in range(_NCORES)
    ]

    trace = os.environ.get("KERNEL_TRACE", "0") == "1"
    res = run_bass_kernel_spmd(nc, in_maps, list(range(_NCORES)), trace=trace)
    LAST_EXEC_TIME_NS = res.exec_time_ns
    LAST_RESULTS = res

    state = np.empty((_B, _Q, 2), np.complex64)
    O = np.empty((_B, 1, 1), np.complex64)
    for c in range(_NCORES):
        o = np.asarray(res.results[c]["outp"], np.float32)  # (128, 4)
        oo = np.asarray(res.results[c]["o_out"], np.float32).reshape(_BL)
        for b in range(_BL):
            st = o[b * 32:b * 32 + _Q, 0:4].reshape(_Q, 2, 2)
            state[c * _BL + b] = st[..., 0] + 1j * st[..., 1]
            O[c * _BL + b, 0, 0] = np.complex64(oo[b])

    return state.reshape(_B, _Q, 1, 2, 1), O
